# revision 30
# baseline (speedup 1.0000x reference)
"""MLA-style attention (decoupled-RoPE) Trainium2 Bass kernel, 8-core SPMD.

Sharding: batch (2) x head-group (4 groups of 4 heads) = 8 cores.
Each core computes its batch's tokens for its 4 heads end-to-end
(projections -> rope -> causal softmax attention -> w_o partial),
returning a partial feature-major [D_MODEL, T] bf16 output; the host sums
the 4 head-group partials per batch element and transposes.

v2: all operands bf16 (f32 PSUM accumulation), single pass over x per
512-token block (two projection sweeps cycling one 8-slot PSUM ring, since
PSUM accumulation groups are bank-granular), q/qr/k/v/k_rope/attention-out
all SBUF-resident (no DRAM staging round-trips), denominator via
ones[128,128] broadcast matmul, diagonal score tiles trimmed to their
unmasked columns, score pipeline prefetched across head boundaries, output
projection interleaved per q-block, bf16 HBM output, DMAs ordered to match
consumption.
"""
import os
import sys

sys.path.insert(0, "/opt/trn_rl_repo")
os.environ.setdefault("JAX_PLATFORMS", "axon")

import numpy as np
import ml_dtypes

import concourse.bacc as bacc
import concourse.mybir as mybir
import concourse.tile as tile
from concourse import bass_utils

# Model constants (hardcoded from the problem spec).
B, T, DM = 2, 2048, 2048
NH, DH, DL, DR = 16, 128, 512, 64
HPC = 4                      # heads per core
GF = HPC * DH                # 512 head-features per core
QRF = HPC * DR               # 256 rope features per core
SCALE = 1.0 / np.sqrt(DH + DR)
ROPE_BASE = 10000.0
N_CORES = 8

F32 = mybir.dt.float32
BF16 = mybir.dt.bfloat16
FP8 = mybir.dt.float8e4
EXP = mybir.ActivationFunctionType.Exp
DRM = mybir.MatmulPerfMode.DoubleRow

TB = 512                     # projection token block
NTB = T // TB                # 4
NKC = DM // 128              # 16 contraction chunks over d_model
NLC = DL // 128              # 4 contraction chunks over d_latent
QBS = 512                    # attention q block
NQB = T // QBS               # 4
DNLAG = 3                    # dn-matmul lag behind the o chain (tiles)
_OPT = {"dnlag": DNLAG, "c_defer": True, "bpt_bufs": 8, "fp8_attn": True}


def build_nc(reps=1, phases="ABC", dnlag=DNLAG, c_defer=True, bpt_bufs=8,
             fp8_attn=True):
    global _OPT
    _OPT = {"dnlag": dnlag, "c_defer": c_defer, "bpt_bufs": bpt_bufs,
            "fp8_attn": fp8_attn}
    nc = bacc.Bacc("TRN2", target_bir_lowering=False, debug=False)

    # External inputs (per-core shards, host-prepared; weights pre-packed
    # into SBUF layout [128, chunk*feat])
    xT = nc.dram_tensor("xT", [DM, T], BF16, kind="ExternalInput").ap()
    wdkv = nc.dram_tensor("wdkv", [128, NKC * DL], BF16, kind="ExternalInput").ap()
    wq = nc.dram_tensor("wq", [128, NKC * GF], BF16, kind="ExternalInput").ap()
    wqr = nc.dram_tensor("wqr", [128, NKC * QRF], BF16, kind="ExternalInput").ap()
    # k_rot weights duplicated across both 64-column halves so k_rope lands
    # duplicated on partitions 0:64 and 64:128 (matmul cost is free-dim only)
    wkr = nc.dram_tensor("wkr", [128, NKC * 128], BF16, kind="ExternalInput").ap()
    wk = nc.dram_tensor("wk", [128, NLC * GF], BF16, kind="ExternalInput").ap()
    wv = nc.dram_tensor("wv", [128, NLC * GF], BF16, kind="ExternalInput").ap()
    wo = nc.dram_tensor("wo", [128, NLC * DM], BF16, kind="ExternalInput").ap()
    cos2 = nc.dram_tensor("cos2", [128, T], F32, kind="ExternalInput").ap()
    ssin2 = nc.dram_tensor("ssin2", [128, T], F32, kind="ExternalInput").ap()
    dmask = nc.dram_tensor("dmask", [128, QBS], BF16, kind="ExternalInput").ap()
    onesd = nc.dram_tensor("onesd", [128, 128], BF16, kind="ExternalInput").ap()

    outPT = nc.dram_tensor("outPT", [DM, T], BF16, kind="ExternalOutput").ap()

    with tile.TileContext(nc) as tc, \
         nc.allow_low_precision(reason="bf16 operands are intentional"):
        with tc.tile_pool(name="gfix", bufs=1) as gfix, \
             tc.tile_pool(name="araw", bufs=3) as araw, \
             tc.tile_pool(name="arp", bufs=2) as arp:
            mask_sb = gfix.tile([128, QBS], BF16, name="mask_sb")
            ones_sb = gfix.tile([128, 128], BF16, name="ones_sb")
            cos_sb = gfix.tile([128, T], F32, name="cos_sb")
            sin_sb = gfix.tile([128, T], F32, name="sin_sb")
            ones8_sb = gfix.tile([128, 256], FP8, name="ones8_sb")
            mask8_sb = gfix.tile([128, QBS], FP8, name="mask8_sb")
            nc.vector.memset(ones8_sb[:], 1.0)

            for _rep in range(reps):
                with tc.tile_pool(name="persist", bufs=1) as pp:
                    kcs = [pp.tile([128, T], BF16, name=f"kc{h}") for h in range(HPC)]
                    vhs = [pp.tile([128, T], BF16, name=f"vh{h}") for h in range(HPC)]
                    v8s = [pp.tile([128, T], FP8, name=f"v8_{h}") for h in range(HPC)]
                    krope = pp.tile([128, T], BF16, name="krope")
                    qsb = [pp.tile([128, T], BF16, name=f"q{h}") for h in range(HPC)]
                    qrsb = [pp.tile([128, T], BF16, name=f"qr{t}") for t in range(2)]
                    osb = [pp.tile([128, T], BF16, name=f"o{h}") for h in range(HPC)]

                    # attention constants
                    nc.gpsimd.dma_start(out=mask_sb[:], in_=dmask)
                    nc.gpsimd.dma_start(out=ones_sb[:], in_=onesd)
                    nc.gpsimd.dma_start(out=cos_sb[:], in_=cos2)
                    nc.gpsimd.dma_start(out=sin_sb[:], in_=ssin2)
                    nc.gpsimd.tensor_copy(mask8_sb[:], mask_sb[:])

                    rope_tail = _phase_a(
                        nc, tc, xT, wdkv, wq, wqr, wkr, wk, wv,
                        cos_sb, sin_sb, kcs, vhs, krope, qsb, qrsb,
                        araw, arp, v8s)
                    if phases == "A":
                        rope_tail()
                        for h in range(HPC):
                            nc.sync.dma_start(
                                out=outPT[h * 128:(h + 1) * 128, :],
                                in_=kcs[h][:])
                            nc.sync.dma_start(
                                out=outPT[512 + h * 128:512 + (h + 1) * 128, :],
                                in_=qsb[h][:])
                            nc.sync.dma_start(
                                out=outPT[1024 + h * 128:1024 + (h + 1) * 128, :],
                                in_=vhs[h][:])
                        nc.sync.dma_start(out=outPT[1536:1664, :], in_=krope[:])
                        nc.sync.dma_start(out=outPT[1664:1792, :], in_=qrsb[0][:])
                        nc.sync.dma_start(out=outPT[1792:1920, :], in_=qrsb[1][:])
                    else:
                        _phase_bc(nc, tc, wo, mask_sb, ones_sb,
                                  kcs, vhs, krope, qsb, qrsb, osb, outPT,
                                  rope_tail, with_c=("C" in phases),
                                  v8s=v8s, ones8_sb=ones8_sb,
                                  mask8_sb=mask8_sb)

    nc.compile()
    return nc


def _phase_a(nc, tc, xT, wdkv, wq, wqr, wkr, wk, wv,
             cos_sb, sin_sb, kcs, vhs, krope, qsb, qrsb, araw, arp, v8s):
    """Single pass over x per 512-token block. PSUM accumulation groups are
    bank-granular, so the 11 projection accumulators run as two sequential
    sweeps (dkv: 4, then q/qr/kr: 7) cycling one 8-slot PSUM ring; k/v
    second-level matmuls reuse the same ring."""
    with (
        tc.tile_pool(name="aw", bufs=1) as aw,
        tc.tile_pool(name="ax", bufs=20) as ax,
        tc.tile_pool(name="adkv", bufs=2) as adkv,
        tc.tile_pool(name="aps", bufs=8, space="PSUM") as aps,
    ):
        wdkv_sb = aw.tile([128, NKC * DL], BF16, name="wdkv_sb")
        wq_sb = aw.tile([128, NKC * GF], BF16, name="wq_sb")
        wqr_sb = aw.tile([128, NKC * QRF], BF16, name="wqr_sb")
        wkr_sb = aw.tile([128, NKC * 128], BF16, name="wkr_sb")
        wk_sb = aw.tile([128, NLC * GF], BF16, name="wk_sb")
        wv_sb = aw.tile([128, NLC * GF], BF16, name="wv_sb")

        xts = {}

        def _g2_chunk(c):
            nc.sync.dma_start(
                out=wq_sb[:, c * 4 * GF:(c + 1) * 4 * GF],
                in_=wq[:, c * 4 * GF:(c + 1) * 4 * GF])
            nc.sync.dma_start(
                out=wqr_sb[:, c * 4 * QRF:(c + 1) * 4 * QRF],
                in_=wqr[:, c * 4 * QRF:(c + 1) * 4 * QRF])
            nc.sync.dma_start(
                out=wkr_sb[:, c * 4 * 128:(c + 1) * 4 * 128],
                in_=wkr[:, c * 4 * 128:(c + 1) * 4 * 128])

        def load_xb(xb, with_weights=False):
            tiles = []
            for ki in range(NKC):
                xt = ax.tile([128, TB], BF16, name="xt", tag="xt")
                nc.sync.dma_start(
                    out=xt[:], in_=xT[ki * 128:(ki + 1) * 128,
                                      xb * TB:(xb + 1) * TB])
                tiles.append(xt)
                if with_weights:
                    # weights ride the x stream in consumption order: wdkv
                    # (sweep 1) front-loaded, then the sweep-2 tensors
                    g1 = {0: (0, 1), 1: (1, 2), 2: (2, 4), 4: (4, 7),
                          6: (7, 10), 8: (10, 13), 10: (13, 16)}
                    if ki in g1:
                        lo, hi = g1[ki]
                        nc.sync.dma_start(
                            out=wdkv_sb[:, lo * DL:hi * DL],
                            in_=wdkv[:, lo * DL:hi * DL])
                    elif ki in (11, 13, 15):
                        _g2_chunk((ki - 11) // 2)
            xts[xb] = tiles
            if with_weights:
                _g2_chunk(3)

        load_xb(0, with_weights=True)
        nc.gpsimd.dma_start(out=wk_sb[:], in_=wk)
        nc.gpsimd.dma_start(out=wv_sb[:], in_=wv)

        def acc_tile():
            return aps.tile([128, TB], F32, name="acc", tag="acc")

        for tb in range(NTB):
            if tb + 1 < NTB:
                load_xb(tb + 1)
            tsl = slice(tb * TB, (tb + 1) * TB)

            # sweep 1: dkv
            acc_dkv = [acc_tile() for _ in range(4)]
            for ki in range(NKC):
                rhs = xts[tb][ki][:]
                st_, sp_ = (ki == 0), (ki == NKC - 1)
                for fi in range(4):
                    nc.tensor.matmul(
                        acc_dkv[fi][:],
                        wdkv_sb[:, ki * DL + fi * 128: ki * DL + (fi + 1) * 128],
                        rhs, start=st_, stop=sp_)
            dkv_sb = adkv.tile([128, NLC * TB], BF16, name="dkv_sb", tag="dkv")
            for fi in range(4):
                dst = dkv_sb[:, fi * TB:(fi + 1) * TB]
                if fi % 2 == 0:
                    nc.vector.tensor_copy(dst, acc_dkv[fi][:])
                else:
                    nc.scalar.copy(dst, acc_dkv[fi][:])

            # sweep 2: q, q_rot, k_rot
            acc_q = [acc_tile() for _ in range(4)]
            acc_qr = [acc_tile() for _ in range(2)]
            acc_kr = acc_tile()
            for ki in range(NKC):
                rhs = xts[tb][ki][:]
                st_, sp_ = (ki == 0), (ki == NKC - 1)
                for fi in range(4):
                    nc.tensor.matmul(
                        acc_q[fi][:],
                        wq_sb[:, ki * GF + fi * 128: ki * GF + (fi + 1) * 128],
                        rhs, start=st_, stop=sp_)
                for fi in range(2):
                    nc.tensor.matmul(
                        acc_qr[fi][:],
                        wqr_sb[:, ki * QRF + fi * 128: ki * QRF + (fi + 1) * 128],
                        rhs, start=st_, stop=sp_)
                nc.tensor.matmul(
                    acc_kr[:], wkr_sb[:, ki * 128:(ki + 1) * 128], rhs,
                    start=st_, stop=sp_)

            # q -> resident bf16 tiles
            for fi in range(4):
                if fi % 2 == 0:
                    nc.vector.tensor_copy(qsb[fi][:, tsl], acc_q[fi][:])
                else:
                    nc.scalar.copy(qsb[fi][:, tsl], acc_q[fi][:])

            # rope stage 1: evacuate the qr/kr accumulators to SBUF now (frees
            # their PSUM slots); the shuffle+mul work is emitted after k/v so
            # no PE wait transitively includes the slow shuffle DMAs
            raws = []
            for acc in (acc_qr[0], acc_qr[1], acc_kr):
                raw = araw.tile([128, TB], F32, name="rraw", tag="rraw")
                nc.scalar.copy(raw[:], acc[:])
                raws.append(raw)

            # second level: k (feature-major) and v (token-major) from dkv.
            # For the last block, v runs first so the PSUM ring's tail is
            # k accumulators (one fast copy each) -- phase B's first score
            # tiles WAR on whichever copies trail A's final matmuls.
            last = tb == NTB - 1

            def emit_k():
                for fi in range(4):
                    kps = acc_tile()
                    for c in range(NLC):
                        nc.tensor.matmul(
                            kps[:],
                            wk_sb[:, c * GF + fi * 128: c * GF + (fi + 1) * 128],
                            dkv_sb[:, c * TB:(c + 1) * TB],
                            start=(c == 0), stop=(c == NLC - 1))
                    if fi % 2 == 0:
                        nc.vector.tensor_copy(kcs[fi][:, tsl], kps[:])
                    else:
                        nc.scalar.copy(kcs[fi][:, tsl], kps[:])

            def emit_v():
                for tt in range(4):
                    vps = acc_tile()
                    for c in range(NLC):
                        nc.tensor.matmul(
                            vps[:],
                            dkv_sb[:, c * TB + tt * 128: c * TB + tt * 128 + 128],
                            wv_sb[:, c * GF:(c + 1) * GF],
                            start=(c == 0), stop=(c == NLC - 1))
                    cc = tb * 4 + tt
                    for h in range(HPC):
                        dst = vhs[h][:, cc * 128:(cc + 1) * 128]
                        src = vps[:, h * 128:(h + 1) * 128]
                        if h % 2 == 0:
                            nc.vector.tensor_copy(dst, src)
                        else:
                            nc.scalar.copy(dst, src)

            if last:
                emit_v()
                emit_k()
            else:
                emit_k()
                emit_v()
            # fp8 shadow of v for phase-B DoubleRow o-matmuls (Pool engine,
            # off every critical path)
            for h in range(HPC):
                nc.gpsimd.tensor_copy(v8s[h][:, tsl], vhs[h][:, tsl])

            # rope stage 2: ro = raw*cos + shuffled(raw)*ssin (ssin rows carry
            # the -sin/+sin signs); the 32-half swap runs as SBUF->SBUF DMAs.
            # The last block's stage 2 is deferred into early phase B: its
            # trailing shuffle DMAs otherwise hold up B's batched sem waits,
            # and B only reads this block's rope at qj=3.
            def rope2(raws=raws, tsl=tsl):
                cs, ss = cos_sb[:, tsl], sin_sb[:, tsl]
                for raw, out_ap in zip(raws, [qrsb[0][:, tsl],
                                              qrsb[1][:, tsl],
                                              krope[:, tsl]]):
                    sh = arp.tile([128, TB], F32, name="rsh", tag="rsh")
                    for bb in range(4):
                        sb_ = bb ^ 1  # swap 32-halves within each 64-block
                        nc.gpsimd.dma_start(out=sh[bb * 32:(bb + 1) * 32, :],
                                            in_=raw[sb_ * 32:(sb_ + 1) * 32, :])
                    m1 = arp.tile([128, TB], F32, name="rm1", tag="rm1")
                    nc.vector.tensor_mul(m1[:], raw[:], cs)
                    m2 = arp.tile([128, TB], F32, name="rm2", tag="rm2")
                    nc.vector.tensor_mul(m2[:], sh[:], ss)
                    nc.vector.tensor_add(out_ap, m1[:], m2[:])

            if tb < NTB - 1:
                rope2()
            else:
                rope_tail = rope2

        return rope_tail


def _phase_bc(nc, tc, wo, mask_sb, ones_sb, kcs, vhs, krope, qsb, qrsb,
              osb, outPT, rope_tail=None, with_c=True, v8s=None,
              ones8_sb=None, mask8_sb=None):
    """Causal attention (q-block outer, head inner) with the output
    projection interleaved per q-block. Everything SBUF-resident."""
    with (
        tc.tile_pool(name="cw", bufs=1) as cw,
        tc.tile_pool(name="bpt", bufs=_OPT["bpt_bufs"]) as bpt,
        tc.tile_pool(name="bpt8", bufs=4) as bpt8,
        tc.tile_pool(name="binv", bufs=2) as binv,
        tc.tile_pool(name="cout", bufs=4) as cout,
        tc.tile_pool(name="psst", bufs=3, space="PSUM") as psst,
        tc.tile_pool(name="pso", bufs=2, space="PSUM") as pso,
        tc.tile_pool(name="psdn", bufs=1, space="PSUM") as psdn,
        tc.tile_pool(name="cps", bufs=2, space="PSUM") as cps,
    ):
        wo_sb = cw.tile([128, NLC * DM], BF16, name="wo_sb")
        # C(qj=0) already contracts over all 4 chunks -> load them all now
        for c in range(NLC):
            nc.gpsimd.dma_start(
                out=wo_sb[:, c * DM:(c + 1) * DM],
                in_=wo[:, c * DM:(c + 1) * DM])

        def col0(qj, ki):
            # diagonal tile i: q-columns < 128*i are fully masked;
            # shrink the free dim instead of multiplying by zeros
            i = ki - 4 * qj
            return 128 * i if i > 0 else 0

        def emit_st(qj, h, ki):
            o = col0(qj, ki)
            qsl = slice(qj * QBS, (qj + 1) * QBS)
            p0 = (h % 2) * 64
            qr = qrsb[h // 2][p0:p0 + 64, qsl]
            st = psst.tile([128, QBS], F32, name="stp", tag="stp")
            nc.tensor.matmul(
                st[:, o:], kcs[h][:, ki * 128:(ki + 1) * 128],
                qsb[h][:, qsl][:, o:], start=True, stop=False)
            nc.tensor.matmul(
                st[:, o:], krope[p0:p0 + 64, ki * 128:(ki + 1) * 128],
                qr[:, o:], start=False, stop=True)
            return st

        def emit_c_chain(qj, di):
            qsl = slice(qj * QBS, (qj + 1) * QBS)
            ps = cps.tile([128, QBS], F32, name="cpst", tag="cpst")
            for c in range(NLC):
                nc.tensor.matmul(
                    ps[:],
                    wo_sb[:, c * DM + di * 128: c * DM + (di + 1) * 128],
                    osb[c][:, qsl],
                    start=(c == 0), stop=(c == NLC - 1))
            co = cout.tile([128, QBS], BF16, name="co", tag="co")
            if di % 2 == 0:
                nc.vector.tensor_copy(co[:], ps[:])
                nc.sync.dma_start(
                    out=outPT[di * 128:(di + 1) * 128, qsl], in_=co[:])
            else:
                nc.scalar.copy(co[:], ps[:])
                nc.scalar.dma_start(
                    out=outPT[di * 128:(di + 1) * 128, qsl], in_=co[:])

        # C chains for q-block qj are deferred into qj+1's head boundaries:
        # they are dependency-free PE work there, filling the in-order PE
        # queue while DVE finishes the new head's reciprocal.
        c_pending = []
        sts = {(0, 0): emit_st(0, 0, 0), (0, 1): emit_st(0, 0, 1)}
        for qj in range(NQB):
            qsl = slice(qj * QBS, (qj + 1) * QBS)
            nk = 4 * (qj + 1)         # causal: tok_k tiles 0..nk-1
            for h in range(HPC):
                vh_sb = vhs[h]
                dn_ps = psdn.tile([128, QBS], F32, name="dn", tag="dn")
                o_ps = pso.tile([128, QBS], F32, name="o", tag="o")

                # dn matmuls lag the o chain by DNLAG tiles: the head's first
                # dn WARs the previous head's reciprocal (psdn has 1 bank);
                # lagging gives the in-order PE queue ~4 tiles of ready work
                # before it reaches that wait.
                pend = []

                def emit_dn(pt, o, ki, nk=nk, dn_ps=dn_ps):
                    nc.tensor.matmul(dn_ps[:, o:], ones_sb[:], pt[:, o:],
                                     start=(ki == 0), stop=(ki == nk - 1))

                # qj>=1: fp8 pt pairs + DoubleRow dn/o matmuls (one DR
                # instruction contracts 2 k-tiles at fp8 rate). qj==0 stays
                # bf16: its short-support rows can't average away fp8 noise.
                use8 = _OPT["fp8_attn"] and qj >= 1
                npairs = nk // 2
                pendp = []

                def emit_pair(p8, oA, pj, npairs=npairs, dn_ps=dn_ps,
                              o_ps=o_ps, h=h):
                    st_, sp_ = (pj == 0), (pj == npairs - 1)
                    pt_ap = p8[:].rearrange("p (two n) -> p two n",
                                            two=2)[:, :, oA:]
                    nc.tensor.matmul(
                        o_ps[:, oA:],
                        v8s[h][:, pj * 256:(pj + 1) * 256].rearrange(
                            "p (two m) -> p two m", two=2),
                        pt_ap, perf_mode=DRM, start=st_, stop=sp_)
                    nc.tensor.matmul(
                        dn_ps[:, oA:],
                        ones8_sb[:].rearrange("p (two m) -> p two m", two=2),
                        pt_ap, perf_mode=DRM, start=st_, stop=sp_)

                p8 = None
                for ki in range(nk):
                    # keep the score pipeline 2 tiles ahead, crossing head
                    # (and q-block) boundaries so exp latency stays hidden
                    if ki + 2 < nk:
                        sts[(h, ki + 2)] = emit_st(qj, h, ki + 2)
                    elif ki + 2 == nk:
                        nh_, nqj = (h + 1, qj) if h + 1 < HPC else (0, qj + 1)
                        if nqj < NQB:
                            sts[(nh_, 0)] = emit_st(nqj, nh_, 0)
                    elif ki + 1 == nk:
                        nh_, nqj = (h + 1, qj) if h + 1 < HPC else (0, qj + 1)
                        if nqj < NQB:
                            sts[(nh_, 1)] = emit_st(nqj, nh_, 1)
                    st = sts.pop((h, ki))
                    o = col0(qj, ki)
                    if not use8:
                        pt = bpt.tile([128, QBS], BF16, name="pt", tag="pt")
                        nc.scalar.activation(pt[:, o:], st[:, o:], EXP,
                                             scale=float(SCALE))
                        if ki - 4 * qj >= 0:  # diagonal tile -> causal mask
                            nc.vector.tensor_mul(pt[:, o:], pt[:, o:],
                                                 mask_sb[:, 0:QBS - o])
                        nc.tensor.matmul(o_ps[:, o:],
                                         vh_sb[:, ki * 128:(ki + 1) * 128],
                                         pt[:, o:],
                                         start=(ki == 0), stop=(ki == nk - 1))
                        pend.append((pt, o, ki))
                        if len(pend) > _OPT["dnlag"]:
                            emit_dn(*pend.pop(0))
                        continue
                    sub, pj = ki % 2, ki // 2
                    if sub == 0:
                        p8 = bpt8.tile([128, 2 * QBS], FP8, name="pt8",
                                       tag="pt8")
                        oA = o
                    base = sub * QBS
                    nc.scalar.activation(p8[:, base + o:base + QBS],
                                         st[:, o:], EXP, scale=float(SCALE))
                    if ki - 4 * qj >= 0:  # diagonal tile -> causal mask
                        nc.vector.tensor_mul(p8[:, base + o:base + QBS],
                                             p8[:, base + o:base + QBS],
                                             mask8_sb[:, 0:QBS - o])
                    if sub == 1:
                        if o > oA:  # zero subtile B's extra masked strip
                            nc.vector.memset(p8[:, QBS + oA:QBS + o], 0.0)
                        pendp.append((p8, oA, pj))
                        if len(pendp) > 1:
                            emit_pair(*pendp.pop(0))
                for args in pend:
                    emit_dn(*args)
                for args in pendp:
                    emit_pair(*args)

                inv = binv.tile([128, QBS], F32, name="inv", tag="inv")
                nc.vector.reciprocal(inv[:], dn_ps[:])
                nc.vector.tensor_mul(osb[h][:, qsl], o_ps[:], inv[:])

                if rope_tail is not None and qj == 0 and h == 0:
                    rope_tail()
                    rope_tail = None

                # fill the head boundary with deferred C chains
                if with_c and _OPT["c_defer"]:
                    for _ in range(min(4, len(c_pending))):
                        emit_c_chain(*c_pending.pop(0))

            if not with_c:
                for h in range(HPC):
                    nc.sync.dma_start(
                        out=outPT[h * 128:(h + 1) * 128, qsl],
                        in_=osb[h][:, qsl])
                continue
            if _OPT["c_defer"]:
                c_pending.extend((qj, di) for di in range(DM // 128))
            else:
                for di in range(DM // 128):
                    emit_c_chain(qj, di)
        for args in c_pending:
            emit_c_chain(*args)


def _pack_w(wT, chunk_rows, feat):
    """[D, F] (contraction-major) -> SBUF layout [128, (D/128)*F]."""
    D = wT.shape[0]
    nck = D // 128
    return np.ascontiguousarray(
        wT.reshape(nck, 128, feat).transpose(1, 0, 2).reshape(128, nck * feat))


def host_prep(x, w_q, w_dkv, w_ukv, w_o, w_q_rot, w_k_rot, mask):
    """Build the 8 per-core input maps (all host-side numpy)."""
    bf = ml_dtypes.bfloat16
    x = np.asarray(x, np.float32)
    w_q = np.asarray(w_q, np.float32)
    w_dkv = np.asarray(w_dkv, np.float32)
    w_ukv = np.asarray(w_ukv, np.float32)
    w_o = np.asarray(w_o, np.float32)
    w_q_rot = np.asarray(w_q_rot, np.float32)
    w_k_rot = np.asarray(w_k_rot, np.float32)

    inv_freq = (1.0 / (ROPE_BASE ** (np.arange(0, DR, 2, dtype=np.float64) / DR)))
    ang = np.arange(T, dtype=np.float64)[:, None] * inv_freq[None, :]   # [T, 32]
    cosb = np.cos(ang).T.astype(np.float32)     # [32, T]
    sinb = np.sin(ang).T.astype(np.float32)
    cos64 = np.vstack([cosb, cosb])
    ssin64 = np.vstack([-sinb, sinb])
    cos2 = np.ascontiguousarray(np.tile(cos64, (2, 1)))     # [128, T]
    ssin2 = np.ascontiguousarray(np.tile(ssin64, (2, 1)))

    r = np.arange(128)[:, None]
    c = np.arange(QBS)[None, :]
    dmask = (r <= c).astype(bf)                 # [128, 512] block-0 mask

    wdkv_p = _pack_w(w_dkv.T.astype(bf), 128, DL)
    # duplicate k_rot features across both 64-column halves
    wkr_dup = np.concatenate([w_k_rot.T, w_k_rot.T], axis=1)  # [DM, 128]
    wkr_p = _pack_w(wkr_dup.astype(bf), 128, 128)
    wukv4 = w_ukv.reshape(NH, 2, DH, DL)
    ones_in = np.ones((128, 128), bf)

    in_maps = []
    for core in range(N_CORES):
        b, g = core // 4, core % 4
        heads = range(4 * g, 4 * g + 4)
        wkT = np.ascontiguousarray(
            np.concatenate([wukv4[h, 0] for h in heads], axis=0).T)  # [DL, GF]
        wvT = np.ascontiguousarray(
            np.concatenate([wukv4[h, 1] for h in heads], axis=0).T)
        in_maps.append({
            "xT": np.ascontiguousarray(x[b].T.astype(bf)),
            "wdkv": wdkv_p,
            "wq": _pack_w(w_q[g * GF:(g + 1) * GF].T.astype(bf), 128, GF),
            "wqr": _pack_w(w_q_rot[g * QRF:(g + 1) * QRF].T.astype(bf), 128, QRF),
            "wkr": wkr_p,
            "wk": _pack_w(wkT.astype(bf), 128, GF),
            "wv": _pack_w(wvT.astype(bf), 128, GF),
            "wo": _pack_w(w_o[:, g * GF:(g + 1) * GF].T.astype(bf), 128, DM),
            "cos2": cos2,
            "ssin2": ssin2,
            "dmask": dmask,
            "onesd": ones_in,
        })
    return in_maps


_NC_CACHE = None


def get_nc():
    global _NC_CACHE
    if _NC_CACHE is None:
        _NC_CACHE = build_nc()
    return _NC_CACHE


def kernel(**inputs) -> np.ndarray:
    nc = get_nc()
    in_maps = host_prep(**inputs)
    res = bass_utils.run_bass_kernel_spmd(nc, in_maps, core_ids=list(range(N_CORES)))
    out = np.zeros((B, DM, T), np.float32)
    for core in range(N_CORES):
        out[core // 4] += res.results[core]["outPT"].astype(np.float32)
    return np.ascontiguousarray(out.transpose(0, 2, 1))



# revision 32
# speedup vs baseline: 8.9463x; 8.9463x over previous
"""MLA-style attention (decoupled-RoPE) Trainium2 Bass kernel, 8-core SPMD.

Sharding: batch (2) x head-group (4 groups of 4 heads) = 8 cores.
Each core computes its batch's tokens for its 4 heads end-to-end
(projections -> rope -> causal softmax attention -> w_o partial),
returning a partial feature-major [D_MODEL, T] bf16 output; the host sums
the 4 head-group partials per batch element and transposes.

v2: all operands bf16 (f32 PSUM accumulation), single pass over x per
512-token block (two projection sweeps cycling one 8-slot PSUM ring, since
PSUM accumulation groups are bank-granular), q/qr/k/v/k_rope/attention-out
all SBUF-resident (no DRAM staging round-trips), denominator via
ones[128,128] broadcast matmul, diagonal score tiles trimmed to their
unmasked columns, score pipeline prefetched across head boundaries, output
projection interleaved per q-block, bf16 HBM output, DMAs ordered to match
consumption.
"""
import os
import sys

sys.path.insert(0, "/opt/trn_rl_repo")
os.environ.setdefault("JAX_PLATFORMS", "axon")

import numpy as np
import ml_dtypes

import concourse.bacc as bacc
import concourse.mybir as mybir
import concourse.tile as tile
from concourse import bass_utils

# Model constants (hardcoded from the problem spec).
B, T, DM = 2, 2048, 2048
NH, DH, DL, DR = 16, 128, 512, 64
HPC = 4                      # heads per core
GF = HPC * DH                # 512 head-features per core
QRF = HPC * DR               # 256 rope features per core
SCALE = 1.0 / np.sqrt(DH + DR)
ROPE_BASE = 10000.0
N_CORES = 8

F32 = mybir.dt.float32
BF16 = mybir.dt.bfloat16
FP8 = mybir.dt.float8e4
EXP = mybir.ActivationFunctionType.Exp
DRM = mybir.MatmulPerfMode.DoubleRow

TB = 512                     # projection token block
NTB = T // TB                # 4
NKC = DM // 128              # 16 contraction chunks over d_model
NLC = DL // 128              # 4 contraction chunks over d_latent
QBS = 512                    # attention q block
NQB = T // QBS               # 4
DNLAG = 3                    # dn-matmul lag behind the o chain (tiles)
_OPT = {"dnlag": DNLAG, "c_defer": True, "bpt_bufs": 8, "fp8_attn": True}


def build_nc(reps=1, phases="ABC", dnlag=DNLAG, c_defer=True, bpt_bufs=8,
             fp8_attn=True):
    global _OPT
    _OPT = {"dnlag": dnlag, "c_defer": c_defer, "bpt_bufs": bpt_bufs,
            "fp8_attn": fp8_attn}
    nc = bacc.Bacc("TRN2", target_bir_lowering=False, debug=False)

    # External inputs (per-core shards, host-prepared; weights pre-packed
    # into SBUF layout [128, chunk*feat])
    xT = nc.dram_tensor("xT", [DM, T], BF16, kind="ExternalInput").ap()
    wdkv = nc.dram_tensor("wdkv", [128, NKC * DL], BF16, kind="ExternalInput").ap()
    wq = nc.dram_tensor("wq", [128, NKC * GF], BF16, kind="ExternalInput").ap()
    wqr = nc.dram_tensor("wqr", [128, NKC * QRF], BF16, kind="ExternalInput").ap()
    # k_rot weights duplicated across both 64-column halves so k_rope lands
    # duplicated on partitions 0:64 and 64:128 (matmul cost is free-dim only)
    wkr = nc.dram_tensor("wkr", [128, NKC * 128], BF16, kind="ExternalInput").ap()
    wk = nc.dram_tensor("wk", [128, NLC * GF], BF16, kind="ExternalInput").ap()
    wv = nc.dram_tensor("wv", [128, NLC * GF], BF16, kind="ExternalInput").ap()
    wo = nc.dram_tensor("wo", [128, NLC * DM], BF16, kind="ExternalInput").ap()
    cos2 = nc.dram_tensor("cos2", [128, T], F32, kind="ExternalInput").ap()
    ssin2 = nc.dram_tensor("ssin2", [128, T], F32, kind="ExternalInput").ap()
    dmask = nc.dram_tensor("dmask", [128, QBS], BF16, kind="ExternalInput").ap()
    onesd = nc.dram_tensor("onesd", [128, 128], BF16, kind="ExternalInput").ap()

    outPT = nc.dram_tensor("outPT", [DM, T], BF16, kind="ExternalOutput").ap()

    with tile.TileContext(nc) as tc, \
         nc.allow_low_precision(reason="bf16 operands are intentional"):
        with tc.tile_pool(name="gfix", bufs=1) as gfix, \
             tc.tile_pool(name="araw", bufs=3) as araw, \
             tc.tile_pool(name="arp", bufs=2) as arp:
            mask_sb = gfix.tile([128, QBS], BF16, name="mask_sb")
            ones_sb = gfix.tile([128, 128], BF16, name="ones_sb")
            cos_sb = gfix.tile([128, T], F32, name="cos_sb")
            sin_sb = gfix.tile([128, T], F32, name="sin_sb")
            ones8_sb = gfix.tile([128, 256], FP8, name="ones8_sb")
            mask8_sb = gfix.tile([128, QBS], FP8, name="mask8_sb")
            nc.vector.memset(ones8_sb[:], 1.0)

            for _rep in range(reps):
                with tc.tile_pool(name="persist", bufs=1) as pp:
                    kcs = [pp.tile([128, T], BF16, name=f"kc{h}") for h in range(HPC)]
                    vhs = [pp.tile([128, T], BF16, name=f"vh{h}") for h in range(HPC)]
                    v8s = [pp.tile([128, T], FP8, name=f"v8_{h}") for h in range(HPC)]
                    krope = pp.tile([128, T], BF16, name="krope")
                    qsb = [pp.tile([128, T], BF16, name=f"q{h}") for h in range(HPC)]
                    qrsb = [pp.tile([128, T], BF16, name=f"qr{t}") for t in range(2)]
                    osb = [pp.tile([128, T], BF16, name=f"o{h}") for h in range(HPC)]

                    # attention constants
                    nc.gpsimd.dma_start(out=mask_sb[:], in_=dmask)
                    nc.gpsimd.dma_start(out=ones_sb[:], in_=onesd)
                    nc.gpsimd.dma_start(out=cos_sb[:], in_=cos2)
                    nc.gpsimd.dma_start(out=sin_sb[:], in_=ssin2)
                    nc.vector.tensor_copy(mask8_sb[:], mask_sb[:])

                    rope_tail = _phase_a(
                        nc, tc, xT, wdkv, wq, wqr, wkr, wk, wv,
                        cos_sb, sin_sb, kcs, vhs, krope, qsb, qrsb,
                        araw, arp, v8s)
                    if phases == "A":
                        rope_tail()
                        for h in range(HPC):
                            nc.sync.dma_start(
                                out=outPT[h * 128:(h + 1) * 128, :],
                                in_=kcs[h][:])
                            nc.sync.dma_start(
                                out=outPT[512 + h * 128:512 + (h + 1) * 128, :],
                                in_=qsb[h][:])
                            nc.sync.dma_start(
                                out=outPT[1024 + h * 128:1024 + (h + 1) * 128, :],
                                in_=vhs[h][:])
                        nc.sync.dma_start(out=outPT[1536:1664, :], in_=krope[:])
                        nc.sync.dma_start(out=outPT[1664:1792, :], in_=qrsb[0][:])
                        nc.sync.dma_start(out=outPT[1792:1920, :], in_=qrsb[1][:])
                    else:
                        _phase_bc(nc, tc, wo, mask_sb, ones_sb,
                                  kcs, vhs, krope, qsb, qrsb, osb, outPT,
                                  rope_tail, with_c=("C" in phases),
                                  v8s=v8s, ones8_sb=ones8_sb,
                                  mask8_sb=mask8_sb)

    nc.compile()
    return nc


def _phase_a(nc, tc, xT, wdkv, wq, wqr, wkr, wk, wv,
             cos_sb, sin_sb, kcs, vhs, krope, qsb, qrsb, araw, arp, v8s):
    """Single pass over x per 512-token block. PSUM accumulation groups are
    bank-granular, so the 11 projection accumulators run as two sequential
    sweeps (dkv: 4, then q/qr/kr: 7) cycling one 8-slot PSUM ring; k/v
    second-level matmuls reuse the same ring."""
    with (
        tc.tile_pool(name="aw", bufs=1) as aw,
        tc.tile_pool(name="ax", bufs=20) as ax,
        tc.tile_pool(name="adkv", bufs=2) as adkv,
        tc.tile_pool(name="aps", bufs=8, space="PSUM") as aps,
    ):
        wdkv_sb = aw.tile([128, NKC * DL], BF16, name="wdkv_sb")
        wq_sb = aw.tile([128, NKC * GF], BF16, name="wq_sb")
        wqr_sb = aw.tile([128, NKC * QRF], BF16, name="wqr_sb")
        wkr_sb = aw.tile([128, NKC * 128], BF16, name="wkr_sb")
        wk_sb = aw.tile([128, NLC * GF], BF16, name="wk_sb")
        wv_sb = aw.tile([128, NLC * GF], BF16, name="wv_sb")

        xts = {}

        def _g2_chunk(c):
            nc.sync.dma_start(
                out=wq_sb[:, c * 4 * GF:(c + 1) * 4 * GF],
                in_=wq[:, c * 4 * GF:(c + 1) * 4 * GF])
            nc.sync.dma_start(
                out=wqr_sb[:, c * 4 * QRF:(c + 1) * 4 * QRF],
                in_=wqr[:, c * 4 * QRF:(c + 1) * 4 * QRF])
            nc.sync.dma_start(
                out=wkr_sb[:, c * 4 * 128:(c + 1) * 4 * 128],
                in_=wkr[:, c * 4 * 128:(c + 1) * 4 * 128])

        def load_xb(xb, with_weights=False):
            tiles = []
            for ki in range(NKC):
                xt = ax.tile([128, TB], BF16, name="xt", tag="xt")
                nc.sync.dma_start(
                    out=xt[:], in_=xT[ki * 128:(ki + 1) * 128,
                                      xb * TB:(xb + 1) * TB])
                tiles.append(xt)
                if with_weights:
                    # weights ride the x stream in consumption order: wdkv
                    # (sweep 1) front-loaded, then the sweep-2 tensors
                    g1 = {0: (0, 1), 1: (1, 2), 2: (2, 4), 4: (4, 7),
                          6: (7, 10), 8: (10, 13), 10: (13, 16)}
                    if ki in g1:
                        lo, hi = g1[ki]
                        nc.sync.dma_start(
                            out=wdkv_sb[:, lo * DL:hi * DL],
                            in_=wdkv[:, lo * DL:hi * DL])
                    elif ki in (11, 13, 15):
                        _g2_chunk((ki - 11) // 2)
            xts[xb] = tiles
            if with_weights:
                _g2_chunk(3)

        load_xb(0, with_weights=True)
        nc.gpsimd.dma_start(out=wk_sb[:], in_=wk)
        nc.gpsimd.dma_start(out=wv_sb[:], in_=wv)

        def acc_tile():
            return aps.tile([128, TB], F32, name="acc", tag="acc")

        for tb in range(NTB):
            if tb + 1 < NTB:
                load_xb(tb + 1)
            tsl = slice(tb * TB, (tb + 1) * TB)

            # sweep 1: dkv
            acc_dkv = [acc_tile() for _ in range(4)]
            for ki in range(NKC):
                rhs = xts[tb][ki][:]
                st_, sp_ = (ki == 0), (ki == NKC - 1)
                for fi in range(4):
                    nc.tensor.matmul(
                        acc_dkv[fi][:],
                        wdkv_sb[:, ki * DL + fi * 128: ki * DL + (fi + 1) * 128],
                        rhs, start=st_, stop=sp_)
            dkv_sb = adkv.tile([128, NLC * TB], BF16, name="dkv_sb", tag="dkv")
            for fi in range(4):
                dst = dkv_sb[:, fi * TB:(fi + 1) * TB]
                if fi % 2 == 0:
                    nc.vector.tensor_copy(dst, acc_dkv[fi][:])
                else:
                    nc.scalar.copy(dst, acc_dkv[fi][:])

            # sweep 2: q, q_rot, k_rot
            acc_q = [acc_tile() for _ in range(4)]
            acc_qr = [acc_tile() for _ in range(2)]
            acc_kr = acc_tile()
            for ki in range(NKC):
                rhs = xts[tb][ki][:]
                st_, sp_ = (ki == 0), (ki == NKC - 1)
                for fi in range(4):
                    nc.tensor.matmul(
                        acc_q[fi][:],
                        wq_sb[:, ki * GF + fi * 128: ki * GF + (fi + 1) * 128],
                        rhs, start=st_, stop=sp_)
                for fi in range(2):
                    nc.tensor.matmul(
                        acc_qr[fi][:],
                        wqr_sb[:, ki * QRF + fi * 128: ki * QRF + (fi + 1) * 128],
                        rhs, start=st_, stop=sp_)
                nc.tensor.matmul(
                    acc_kr[:], wkr_sb[:, ki * 128:(ki + 1) * 128], rhs,
                    start=st_, stop=sp_)

            # q -> resident bf16 tiles
            for fi in range(4):
                if fi % 2 == 0:
                    nc.vector.tensor_copy(qsb[fi][:, tsl], acc_q[fi][:])
                else:
                    nc.scalar.copy(qsb[fi][:, tsl], acc_q[fi][:])

            # rope stage 1: evacuate the qr/kr accumulators to SBUF now (frees
            # their PSUM slots); the shuffle+mul work is emitted after k/v so
            # no PE wait transitively includes the slow shuffle DMAs
            raws = []
            for acc in (acc_qr[0], acc_qr[1], acc_kr):
                raw = araw.tile([128, TB], F32, name="rraw", tag="rraw")
                nc.scalar.copy(raw[:], acc[:])
                raws.append(raw)

            # second level: k (feature-major) and v (token-major) from dkv.
            # For the last block, v runs first so the PSUM ring's tail is
            # k accumulators (one fast copy each) -- phase B's first score
            # tiles WAR on whichever copies trail A's final matmuls.
            last = tb == NTB - 1

            def emit_k():
                for fi in range(4):
                    kps = acc_tile()
                    for c in range(NLC):
                        nc.tensor.matmul(
                            kps[:],
                            wk_sb[:, c * GF + fi * 128: c * GF + (fi + 1) * 128],
                            dkv_sb[:, c * TB:(c + 1) * TB],
                            start=(c == 0), stop=(c == NLC - 1))
                    if fi % 2 == 0:
                        nc.vector.tensor_copy(kcs[fi][:, tsl], kps[:])
                    else:
                        nc.scalar.copy(kcs[fi][:, tsl], kps[:])

            def emit_v():
                for tt in range(4):
                    vps = acc_tile()
                    for c in range(NLC):
                        nc.tensor.matmul(
                            vps[:],
                            dkv_sb[:, c * TB + tt * 128: c * TB + tt * 128 + 128],
                            wv_sb[:, c * GF:(c + 1) * GF],
                            start=(c == 0), stop=(c == NLC - 1))
                    cc = tb * 4 + tt
                    for h in range(HPC):
                        dst = vhs[h][:, cc * 128:(cc + 1) * 128]
                        src = vps[:, h * 128:(h + 1) * 128]
                        if h % 2 == 0:
                            nc.vector.tensor_copy(dst, src)
                        else:
                            nc.scalar.copy(dst, src)

            if last:
                emit_v()
                emit_k()
            else:
                emit_k()
                emit_v()
            # fp8 shadow of v for phase-B DoubleRow o-matmuls
            for h in range(HPC):
                if h % 2 == 0:
                    nc.vector.tensor_copy(v8s[h][:, tsl], vhs[h][:, tsl])
                else:
                    nc.scalar.copy(v8s[h][:, tsl], vhs[h][:, tsl])

            # rope stage 2: ro = raw*cos + shuffled(raw)*ssin (ssin rows carry
            # the -sin/+sin signs); the 32-half swap runs as SBUF->SBUF DMAs.
            # The last block's stage 2 is deferred into early phase B: its
            # trailing shuffle DMAs otherwise hold up B's batched sem waits,
            # and B only reads this block's rope at qj=3.
            def rope2(raws=raws, tsl=tsl):
                cs, ss = cos_sb[:, tsl], sin_sb[:, tsl]
                for raw, out_ap in zip(raws, [qrsb[0][:, tsl],
                                              qrsb[1][:, tsl],
                                              krope[:, tsl]]):
                    sh = arp.tile([128, TB], F32, name="rsh", tag="rsh")
                    for bb in range(4):
                        sb_ = bb ^ 1  # swap 32-halves within each 64-block
                        nc.gpsimd.dma_start(out=sh[bb * 32:(bb + 1) * 32, :],
                                            in_=raw[sb_ * 32:(sb_ + 1) * 32, :])
                    m1 = arp.tile([128, TB], F32, name="rm1", tag="rm1")
                    nc.vector.tensor_mul(m1[:], raw[:], cs)
                    m2 = arp.tile([128, TB], F32, name="rm2", tag="rm2")
                    nc.vector.tensor_mul(m2[:], sh[:], ss)
                    nc.vector.tensor_add(out_ap, m1[:], m2[:])

            if tb < NTB - 1:
                rope2()
            else:
                rope_tail = rope2

        return rope_tail


def _phase_bc(nc, tc, wo, mask_sb, ones_sb, kcs, vhs, krope, qsb, qrsb,
              osb, outPT, rope_tail=None, with_c=True, v8s=None,
              ones8_sb=None, mask8_sb=None):
    """Causal attention (q-block outer, head inner) with the output
    projection interleaved per q-block. Everything SBUF-resident."""
    with (
        tc.tile_pool(name="cw", bufs=1) as cw,
        tc.tile_pool(name="bpt", bufs=_OPT["bpt_bufs"]) as bpt,
        tc.tile_pool(name="bpt8", bufs=4) as bpt8,
        tc.tile_pool(name="binv", bufs=2) as binv,
        tc.tile_pool(name="cout", bufs=4) as cout,
        tc.tile_pool(name="psst", bufs=3, space="PSUM") as psst,
        tc.tile_pool(name="pso", bufs=2, space="PSUM") as pso,
        tc.tile_pool(name="psdn", bufs=1, space="PSUM") as psdn,
        tc.tile_pool(name="cps", bufs=2, space="PSUM") as cps,
    ):
        wo_sb = cw.tile([128, NLC * DM], BF16, name="wo_sb")
        # C(qj=0) already contracts over all 4 chunks -> load them all now
        for c in range(NLC):
            nc.gpsimd.dma_start(
                out=wo_sb[:, c * DM:(c + 1) * DM],
                in_=wo[:, c * DM:(c + 1) * DM])

        def col0(qj, ki):
            # diagonal tile i: q-columns < 128*i are fully masked;
            # shrink the free dim instead of multiplying by zeros
            i = ki - 4 * qj
            return 128 * i if i > 0 else 0

        def emit_st(qj, h, ki):
            o = col0(qj, ki)
            qsl = slice(qj * QBS, (qj + 1) * QBS)
            p0 = (h % 2) * 64
            qr = qrsb[h // 2][p0:p0 + 64, qsl]
            st = psst.tile([128, QBS], F32, name="stp", tag="stp")
            nc.tensor.matmul(
                st[:, o:], kcs[h][:, ki * 128:(ki + 1) * 128],
                qsb[h][:, qsl][:, o:], start=True, stop=False)
            nc.tensor.matmul(
                st[:, o:], krope[p0:p0 + 64, ki * 128:(ki + 1) * 128],
                qr[:, o:], start=False, stop=True)
            return st

        def emit_c_chain(qj, di):
            qsl = slice(qj * QBS, (qj + 1) * QBS)
            ps = cps.tile([128, QBS], F32, name="cpst", tag="cpst")
            for c in range(NLC):
                nc.tensor.matmul(
                    ps[:],
                    wo_sb[:, c * DM + di * 128: c * DM + (di + 1) * 128],
                    osb[c][:, qsl],
                    start=(c == 0), stop=(c == NLC - 1))
            co = cout.tile([128, QBS], BF16, name="co", tag="co")
            if di % 2 == 0:
                nc.vector.tensor_copy(co[:], ps[:])
                nc.sync.dma_start(
                    out=outPT[di * 128:(di + 1) * 128, qsl], in_=co[:])
            else:
                nc.scalar.copy(co[:], ps[:])
                nc.scalar.dma_start(
                    out=outPT[di * 128:(di + 1) * 128, qsl], in_=co[:])

        # C chains for q-block qj are deferred into qj+1's head boundaries:
        # they are dependency-free PE work there, filling the in-order PE
        # queue while DVE finishes the new head's reciprocal.
        c_pending = []
        sts = {(0, 0): emit_st(0, 0, 0), (0, 1): emit_st(0, 0, 1)}
        for qj in range(NQB):
            qsl = slice(qj * QBS, (qj + 1) * QBS)
            nk = 4 * (qj + 1)         # causal: tok_k tiles 0..nk-1
            for h in range(HPC):
                vh_sb = vhs[h]
                dn_ps = psdn.tile([128, QBS], F32, name="dn", tag="dn")
                o_ps = pso.tile([128, QBS], F32, name="o", tag="o")

                # dn matmuls lag the o chain by DNLAG tiles: the head's first
                # dn WARs the previous head's reciprocal (psdn has 1 bank);
                # lagging gives the in-order PE queue ~4 tiles of ready work
                # before it reaches that wait.
                pend = []

                def emit_dn(pt, o, ki, nk=nk, dn_ps=dn_ps):
                    nc.tensor.matmul(dn_ps[:, o:], ones_sb[:], pt[:, o:],
                                     start=(ki == 0), stop=(ki == nk - 1))

                # qj>=1: fp8 pt pairs + DoubleRow dn/o matmuls (one DR
                # instruction contracts 2 k-tiles at fp8 rate). qj==0 stays
                # bf16: its short-support rows can't average away fp8 noise.
                use8 = _OPT["fp8_attn"] and qj >= 1
                npairs = nk // 2
                pendp = []

                def emit_pair(p8, oA, pj, npairs=npairs, dn_ps=dn_ps,
                              o_ps=o_ps, h=h):
                    st_, sp_ = (pj == 0), (pj == npairs - 1)
                    pt_ap = p8[:].rearrange("p (two n) -> p two n",
                                            two=2)[:, :, oA:]
                    nc.tensor.matmul(
                        o_ps[:, oA:],
                        v8s[h][:, pj * 256:(pj + 1) * 256].rearrange(
                            "p (two m) -> p two m", two=2),
                        pt_ap, perf_mode=DRM, start=st_, stop=sp_)
                    nc.tensor.matmul(
                        dn_ps[:, oA:],
                        ones8_sb[:].rearrange("p (two m) -> p two m", two=2),
                        pt_ap, perf_mode=DRM, start=st_, stop=sp_)

                p8 = None
                for ki in range(nk):
                    # keep the score pipeline 2 tiles ahead, crossing head
                    # (and q-block) boundaries so exp latency stays hidden
                    if ki + 2 < nk:
                        sts[(h, ki + 2)] = emit_st(qj, h, ki + 2)
                    elif ki + 2 == nk:
                        nh_, nqj = (h + 1, qj) if h + 1 < HPC else (0, qj + 1)
                        if nqj < NQB:
                            sts[(nh_, 0)] = emit_st(nqj, nh_, 0)
                    elif ki + 1 == nk:
                        nh_, nqj = (h + 1, qj) if h + 1 < HPC else (0, qj + 1)
                        if nqj < NQB:
                            sts[(nh_, 1)] = emit_st(nqj, nh_, 1)
                    st = sts.pop((h, ki))
                    o = col0(qj, ki)
                    if not use8:
                        pt = bpt.tile([128, QBS], BF16, name="pt", tag="pt")
                        nc.scalar.activation(pt[:, o:], st[:, o:], EXP,
                                             scale=float(SCALE))
                        if ki - 4 * qj >= 0:  # diagonal tile -> causal mask
                            nc.vector.tensor_mul(pt[:, o:], pt[:, o:],
                                                 mask_sb[:, 0:QBS - o])
                        nc.tensor.matmul(o_ps[:, o:],
                                         vh_sb[:, ki * 128:(ki + 1) * 128],
                                         pt[:, o:],
                                         start=(ki == 0), stop=(ki == nk - 1))
                        pend.append((pt, o, ki))
                        if len(pend) > _OPT["dnlag"]:
                            emit_dn(*pend.pop(0))
                        continue
                    sub, pj = ki % 2, ki // 2
                    if sub == 0:
                        p8 = bpt8.tile([128, 2 * QBS], FP8, name="pt8",
                                       tag="pt8")
                        oA = o
                    base = sub * QBS
                    nc.scalar.activation(p8[:, base + o:base + QBS],
                                         st[:, o:], EXP, scale=float(SCALE))
                    if ki - 4 * qj >= 0:  # diagonal tile -> causal mask
                        nc.vector.tensor_mul(p8[:, base + o:base + QBS],
                                             p8[:, base + o:base + QBS],
                                             mask8_sb[:, 0:QBS - o])
                    if sub == 1:
                        if o > oA:  # zero subtile B's extra masked strip
                            nc.vector.memset(p8[:, QBS + oA:QBS + o], 0.0)
                        pendp.append((p8, oA, pj))
                        if len(pendp) > 1:
                            emit_pair(*pendp.pop(0))
                for args in pend:
                    emit_dn(*args)
                for args in pendp:
                    emit_pair(*args)

                inv = binv.tile([128, QBS], F32, name="inv", tag="inv")
                nc.vector.reciprocal(inv[:], dn_ps[:])
                nc.vector.tensor_mul(osb[h][:, qsl], o_ps[:], inv[:])

                if rope_tail is not None and qj == 0 and h == 0:
                    rope_tail()
                    rope_tail = None

                # fill the head boundary with deferred C chains
                if with_c and _OPT["c_defer"]:
                    for _ in range(min(4, len(c_pending))):
                        emit_c_chain(*c_pending.pop(0))

            if not with_c:
                for h in range(HPC):
                    nc.sync.dma_start(
                        out=outPT[h * 128:(h + 1) * 128, qsl],
                        in_=osb[h][:, qsl])
                continue
            if _OPT["c_defer"]:
                c_pending.extend((qj, di) for di in range(DM // 128))
            else:
                for di in range(DM // 128):
                    emit_c_chain(qj, di)
        for args in c_pending:
            emit_c_chain(*args)


def _pack_w(wT, chunk_rows, feat):
    """[D, F] (contraction-major) -> SBUF layout [128, (D/128)*F]."""
    D = wT.shape[0]
    nck = D // 128
    return np.ascontiguousarray(
        wT.reshape(nck, 128, feat).transpose(1, 0, 2).reshape(128, nck * feat))


def host_prep(x, w_q, w_dkv, w_ukv, w_o, w_q_rot, w_k_rot, mask):
    """Build the 8 per-core input maps (all host-side numpy)."""
    bf = ml_dtypes.bfloat16
    x = np.asarray(x, np.float32)
    w_q = np.asarray(w_q, np.float32)
    w_dkv = np.asarray(w_dkv, np.float32)
    w_ukv = np.asarray(w_ukv, np.float32)
    w_o = np.asarray(w_o, np.float32)
    w_q_rot = np.asarray(w_q_rot, np.float32)
    w_k_rot = np.asarray(w_k_rot, np.float32)

    inv_freq = (1.0 / (ROPE_BASE ** (np.arange(0, DR, 2, dtype=np.float64) / DR)))
    ang = np.arange(T, dtype=np.float64)[:, None] * inv_freq[None, :]   # [T, 32]
    cosb = np.cos(ang).T.astype(np.float32)     # [32, T]
    sinb = np.sin(ang).T.astype(np.float32)
    cos64 = np.vstack([cosb, cosb])
    ssin64 = np.vstack([-sinb, sinb])
    cos2 = np.ascontiguousarray(np.tile(cos64, (2, 1)))     # [128, T]
    ssin2 = np.ascontiguousarray(np.tile(ssin64, (2, 1)))

    r = np.arange(128)[:, None]
    c = np.arange(QBS)[None, :]
    dmask = (r <= c).astype(bf)                 # [128, 512] block-0 mask

    wdkv_p = _pack_w(w_dkv.T.astype(bf), 128, DL)
    # duplicate k_rot features across both 64-column halves
    wkr_dup = np.concatenate([w_k_rot.T, w_k_rot.T], axis=1)  # [DM, 128]
    wkr_p = _pack_w(wkr_dup.astype(bf), 128, 128)
    wukv4 = w_ukv.reshape(NH, 2, DH, DL)
    ones_in = np.ones((128, 128), bf)

    in_maps = []
    for core in range(N_CORES):
        b, g = core // 4, core % 4
        heads = range(4 * g, 4 * g + 4)
        wkT = np.ascontiguousarray(
            np.concatenate([wukv4[h, 0] for h in heads], axis=0).T)  # [DL, GF]
        wvT = np.ascontiguousarray(
            np.concatenate([wukv4[h, 1] for h in heads], axis=0).T)
        in_maps.append({
            "xT": np.ascontiguousarray(x[b].T.astype(bf)),
            "wdkv": wdkv_p,
            "wq": _pack_w(w_q[g * GF:(g + 1) * GF].T.astype(bf), 128, GF),
            "wqr": _pack_w(w_q_rot[g * QRF:(g + 1) * QRF].T.astype(bf), 128, QRF),
            "wkr": wkr_p,
            "wk": _pack_w(wkT.astype(bf), 128, GF),
            "wv": _pack_w(wvT.astype(bf), 128, GF),
            "wo": _pack_w(w_o[:, g * GF:(g + 1) * GF].T.astype(bf), 128, DM),
            "cos2": cos2,
            "ssin2": ssin2,
            "dmask": dmask,
            "onesd": ones_in,
        })
    return in_maps


_NC_CACHE = None


def get_nc():
    global _NC_CACHE
    if _NC_CACHE is None:
        _NC_CACHE = build_nc()
    return _NC_CACHE


def kernel(**inputs) -> np.ndarray:
    nc = get_nc()
    in_maps = host_prep(**inputs)
    res = bass_utils.run_bass_kernel_spmd(nc, in_maps, core_ids=list(range(N_CORES)))
    out = np.zeros((B, DM, T), np.float32)
    for core in range(N_CORES):
        out[core // 4] += res.results[core]["outPT"].astype(np.float32)
    return np.ascontiguousarray(out.transpose(0, 2, 1))

